# revision 34
# baseline (speedup 1.0000x reference)
"""BoxFuse (sparse_attention) Trainium2 Bass kernel.

Data-parallel over batch: 32 batches -> 8 NeuronCores x 4 batches.

Structure (baseline 75.7us -> ~47us):
  - Wqk = Wk_eff @ (Wq_eff*scale)^T folded on HOST [1536, 1024]: the device
    computes cT[f, l-pad] = Wqk^T @ boxT directly (att[n,l] = x_hat_v . c).
    This removes the K projection and kT entirely (-29k PE cycles); the kb
    bias enters via cqb = Wq_s @ kb during the cT PSUM->SBUF cast.
  - fp8e4 DoubleRow matmuls for the two big projections (cT and V): weights
    pre-scaled by powers of 2 on host (wqk sigma ~4e-4 underflows fp8e4
    min-subnormal 2^-9 otherwise) and un-scaled in the PSUM casts; the
    transposed box activations are cast fp16->fp8 on DVE after the XBAR
    transpose (XBAR requires 2-byte dtypes). ~2x stream rate on 46k cycles.
  - All 16-bit tensors are fp16 (same PE/DVE speed as bf16 on TRN2, 8x
    lower quantization error). MASK_NEG is -15, not -30: e^-30 flushes to
    zero in fp16 which would NaN the all-masked (length==0) batches; e^-15
    is a representable subnormal and all-masked rows still quantize to one
    uniform value, preserving the reference's uniform-softmax behavior.
  - Box rows are padded 100->112 (XBAR needs rows%16==0) with DUPLICATED
    real rows, never stale SBUF: stale bits become Inf/NaN in fp8 and have
    been observed to wedge the PE exec unit in DoubleRow mode.
  - Weights + consts are hoisted OUTSIDE the reps loop: resident in SBUF
    across reps (steady-state per-rep DMA ~10.6 MB: vit+box in, out back).
  - qb == 0 on the harness inputs, so the exp bias is just the mask bias
    mbs (a general has_qb path adds per-batch wkqb matvecs).
  - All activation transposes run on the DMA engines (dma_start_transpose,
    SBUF->SBUF, no HBM traffic); v bias folded into the vit residual on
    host; outputs written fp16, cast to f32 on host.
"""

import os
import numpy as np

if os.environ.get("JAX_PLATFORMS", "").strip() == "cpu":
    os.environ.pop("JAX_PLATFORMS")

B, NTOK, L, LOW, HIGH = 32, 576, 100, 1024, 1536
NCORES = 8
BPC = B // NCORES            # batches per core
LN_EPS = 1e-5
MASK_NEG = -15.0
HT_HIGH = HIGH // 128        # 12 h-tiles for box features
HT_LOW = LOW // 128          # 8 f-tiles for vit features
DT = LOW // 128              # 8 d-tiles of projected features
LP = 112                     # box tokens padded to XBAR row granularity
LBP = BPC * LP               # 448 padded batch-concat box token dim

_CACHE = {}

# fp8 weight pre-scales (powers of 2; un-scaled in the PSUM->SBUF casts).
# wqk entries have sigma ~4e-4 << fp8e4 min subnormal 2^-9, so they must be
# scaled up into the normal range; same for vw (sigma ~0.02).
S_WQK = 4096.0
S_VW = 128.0


def _flags():
    c8 = os.environ.get("BF_C8", "1") == "1"
    v8 = os.environ.get("BF_V8", "1") == "1"
    return c8, v8


def _build(reps=1, has_qb=False, c8=None, v8=None):
    ec8, ev8 = _flags()
    c8 = ec8 if c8 is None else c8
    v8 = ev8 if v8 is None else v8
    import concourse.bacc as bacc
    import concourse.tile as tile
    import concourse.mybir as mybir

    F32 = mybir.dt.float32
    BF16 = mybir.dt.bfloat16
    F16 = mybir.dt.float16
    FP8 = mybir.dt.float8e4
    AF = mybir.ActivationFunctionType
    ALU = mybir.AluOpType
    DR = mybir.MatmulPerfMode.DoubleRow

    nc = bacc.Bacc("TRN2", target_bir_lowering=False, debug=False)

    vit_d = nc.dram_tensor("vit", [BPC, NTOK, LOW], F16, kind="ExternalInput").ap()
    box_d = nc.dram_tensor("box", [BPC, L, HIGH], F16, kind="ExternalInput").ap()
    wqk_d = nc.dram_tensor("wqk", [128, HT_HIGH, LOW], FP8 if c8 else F16,
                           kind="ExternalInput").ap()
    vw_d = nc.dram_tensor("vw", [128, HT_HIGH, LOW], FP8 if v8 else F16,
                          kind="ExternalInput").ap()
    cqb_d = nc.dram_tensor("cqb", [128, DT], F32, kind="ExternalInput").ap()
    msc_d = nc.dram_tensor("msc", [L, BPC], F32, kind="ExternalInput").ap()
    mbs_d = nc.dram_tensor("mbs", [L, BPC], F32, kind="ExternalInput").ap()
    ones_d = nc.dram_tensor("ones", [128, 1], F16, kind="ExternalInput").ap()
    if has_qb:
        wkqb_d = nc.dram_tensor("wkqb", [128, HT_HIGH], F16,
                                kind="ExternalInput").ap()
    out_d = nc.dram_tensor("out", [BPC, NTOK, LOW], F16, kind="ExternalOutput").ap()

    NT = [(t * 128, min(128, NTOK - t * 128)) for t in range(5)]
    CNT = [(0, 512), (512, 64)]          # att/epilogue free-dim chunks over n
    VNT = [(0, 512), (512, 512)]         # v natural d chunks

    with tile.TileContext(nc) as tc:
      with (
          tc.tile_pool(name="consts", bufs=1) as consts,
          tc.tile_pool(name="wpool", bufs=1) as wpool,
          tc.tile_pool(name="persist", bufs=1) as persist,
          tc.tile_pool(name="small", bufs=int(os.environ.get("BF_SM", "6"))) as small,
          tc.tile_pool(name="pp_mm", bufs=int(os.environ.get("BF_MM", "3")), space="PSUM") as pp_mm,
          tc.tile_pool(name="pp_att", bufs=int(os.environ.get("BF_ATT", "5")), space="PSUM") as pp_att,
          tc.tile_pool(name="xTp", bufs=4) as xTp,
          tc.tile_pool(name="stageB", bufs=int(os.environ.get("BF_SB", "4"))) as stageB,
          tc.tile_pool(name="stageA", bufs=int(os.environ.get("BF_SA", "3"))) as stageA,
          tc.tile_pool(name="boxTp", bufs=int(os.environ.get("BF_BT", "2"))) as boxTp,
          tc.tile_pool(name="attp", bufs=int(os.environ.get("BF_ATTP", "2"))) as attp,
          tc.tile_pool(name="outp", bufs=int(os.environ.get("BF_OUT", "3"))) as outp,
      ):
        # ---- constants + weights: loaded once, resident across reps ----
        ones = consts.tile([128, 1], F16, tag="ones")
        nc.sync.dma_start(ones[:], ones_d)
        msc = consts.tile([128, BPC], F32, tag="msc")
        nc.sync.dma_start(msc[:L, :], msc_d)
        mbs = consts.tile([128, BPC], F32, tag="mbs")
        nc.sync.dma_start(mbs[:L, :], mbs_d)
        cqb = consts.tile([128, DT], F32, tag="cqb")
        nc.sync.dma_start(cqb[:], cqb_d)
        eps_t = consts.tile([128, 1], F32, tag="eps")
        nc.vector.memset(eps_t[:], LN_EPS)
        wqk = wpool.tile([128, HT_HIGH, LOW], FP8 if c8 else F16, tag="wqk")
        nc.scalar.dma_start(wqk[:], wqk_d[:])
        vw = wpool.tile([128, HT_HIGH, LOW], FP8 if v8 else F16, tag="vw")
        nc.scalar.dma_start(vw[:], vw_d[:])
        if has_qb:
            wkqb = consts.tile([128, HT_HIGH], F16, tag="wkqb")
            nc.sync.dma_start(wkqb[:], wkqb_d)

        def layernorm_stats(x_ap, rows, width):
            """x_ap [rows, width] bf16 -> (r, nmr) [rows, 1] f32."""
            chunks = width // 512
            st6 = small.tile([128, chunks, 6], F32, tag="st6")
            for c in range(chunks):
                nc.vector.bn_stats(
                    st6[:rows, c, :], x_ap[:rows, c * 512:(c + 1) * 512]
                )
            st2 = small.tile([128, 2], F32, tag="st2")
            nc.vector.bn_aggr(st2[:rows, :], st6[:rows, :, :])
            sd = small.tile([128, 1], F32, tag="sd")
            nc.scalar.activation(sd[:rows, :], st2[:rows, 1:2], AF.Sqrt,
                                 bias=eps_t[:rows, :], scale=1.0)
            r = small.tile([128, 1], F32, tag="r")
            nc.vector.reciprocal(r[:rows, :], sd[:rows, :])
            nmr = small.tile([128, 1], F32, tag="nmr")
            nc.vector.scalar_tensor_tensor(
                nmr[:rows, :], st2[:rows, 0:1], -1.0, r[:rows, :],
                op0=ALU.mult, op1=ALU.mult,
            )
            return r, nmr

        for _rep in range(reps):
            # ---------------- Phase A ----------------
            # box first on SP (it gates the whole phase-A PE pipeline);
            # vit prefetches ride the DVE ring.
            bxs = []
            for b in range(BPC):
                bx = stageA.tile([128, HIGH], F16, tag="bx",
                                 name="bx", bufs=BPC)
                nc.sync.dma_start(bx[:L, :], box_d[b])
                # pad rows L..LP with duplicated real rows: keeps the whole
                # 112-row transpose pipeline defined (no stale Inf/NaN bits
                # reaching the fp8 cast); the padded columns of cT/vnat are
                # never read.
                nc.sync.dma_start(bx[L:LP, :], box_d[b, 0:LP - L, :])
                bxs.append(bx)
            # vit tiles resident: prefetched during phase A, reread by the
            # epilogue residual add
            vit_all = persist.tile([128, BPC, 5, LOW], F16, tag="vitall")
            for b in range(BPC):
                for t, (st, w) in enumerate(NT):
                    nc.scalar.dma_start(vit_all[:w, b, t, :],
                                        vit_d[b, st:st + w, :])

            vnat = persist.tile([128, BPC, LOW], F16, tag="v",
                                bufs=int(os.environ.get("BF_VN", "2")))
            cT = persist.tile([128, HT_LOW, LBP], F16, tag="cT",
                              bufs=int(os.environ.get("BF_CT", "2")))
            any8 = c8 or v8
            boxT16 = boxTp.tile([128, HT_HIGH, LBP], F16, tag="boxT16",
                                bufs=int(os.environ.get("BF_BT16", "2")))
            if any8:
                boxT8 = boxTp.tile([128, HT_HIGH, LBP], FP8, tag="boxT8",
                                   bufs=2)
            if has_qb:
                bias_all = persist.tile([128, BPC], F32, tag="biasall")

            for b in range(BPC):
                bx = bxs[b]
                r, nmr = layernorm_stats(bx, LP, HIGH)
                xh = stageA.tile([128, HIGH], F16, tag="xhb", name="xh")
                nc.gpsimd.tensor_scalar(xh[:LP, :], bx[:LP, :], r[:LP, :],
                                        nmr[:LP, :], op0=ALU.mult, op1=ALU.add)
                nc.sync.dma_start_transpose(
                    boxT16[:, :, b * LP:(b + 1) * LP], xh[:LP, :]
                )
                if any8:
                    # fp16 -> fp8e4 cast of this batch's transposed columns
                    cast_eng = (nc.gpsimd
                                if os.environ.get("BF_C8ENG", "0") == "1"
                                else nc.vector)
                    cast_eng.tensor_scalar(
                        boxT8[:, :, b * LP:(b + 1) * LP],
                        boxT16[:, :, b * LP:(b + 1) * LP],
                        1.0, None, op0=ALU.mult)
                # V projection for this batch, natural [l, d] layout
                if v8 and os.environ.get("BF_VORD", "0") == "1":
                    # hp-outer: consecutive matmuls share the same stationary
                    # operand (boxT8 slice); two PSUM accumulators stay open
                    # across the contraction loop.
                    pss_v = [pp_mm.tile([128, 512], F32, tag="mm", name="psv")
                             for _ in VNT]
                    for hp in range(HT_HIGH // 2):
                        for ci, (d0, dw) in enumerate(VNT):
                            nc.tensor.matmul(
                                pss_v[ci][:L, :dw],
                                boxT8[:, 2 * hp:2 * hp + 2, b * LP:b * LP + L],
                                vw[:, 2 * hp:2 * hp + 2, d0:d0 + dw],
                                start=(hp == 0), stop=(hp == HT_HIGH // 2 - 1),
                                perf_mode=DR,
                            )
                    for ci, (d0, dw) in enumerate(VNT):
                        nc.scalar.activation(vnat[:L, b, d0:d0 + dw],
                                             pss_v[ci][:L, :dw],
                                             AF.Identity, scale=1.0 / S_VW)
                else:
                    for d0, dw in VNT:
                        ps = pp_mm.tile([128, 512], F32, tag="mm", name="ps")
                        if v8:
                            for hp in range(HT_HIGH // 2):
                                nc.tensor.matmul(
                                    ps[:L, :dw],
                                    boxT8[:, 2 * hp:2 * hp + 2,
                                          b * LP:b * LP + L],
                                    vw[:, 2 * hp:2 * hp + 2, d0:d0 + dw],
                                    start=(hp == 0),
                                    stop=(hp == HT_HIGH // 2 - 1),
                                    perf_mode=DR,
                                )
                        else:
                            for h in range(HT_HIGH):
                                nc.tensor.matmul(
                                    ps[:L, :dw], boxT16[:, h, b * LP:b * LP + L],
                                    vw[:, h, d0:d0 + dw],
                                    start=(h == 0), stop=(h == HT_HIGH - 1),
                                )
                        nc.scalar.activation(vnat[:L, b, d0:d0 + dw],
                                             ps[:L, :dw], AF.Identity,
                                             scale=(1.0 / S_VW) if v8 else 1.0)

            # x_hat_v transpose pipeline for ALL batches: vector/gpsimd/SP
            # flow during the cT PE work below (xTs consumed in phase B).
            # The sqrt/reciprocal/stt tails for the four full 128-row tiles
            # are batched into [128, 4] ops (one instr instead of four).
            xTs = []
            ln_batched = os.environ.get("BF_LNB", "0") == "1"
            for b in range(BPC):
                xT = xTp.tile([128, HT_LOW, NTOK], F16, tag="xT", name="xT")
                xTs.append(xT)
                if ln_batched:
                    st2a = small.tile([128, 4, 2], F32, tag="st2a")
                    for t in range(4):
                        st6 = small.tile([128, 2, 6], F32, tag="st6")
                        for c in range(2):
                            nc.vector.bn_stats(
                                st6[:, c, :],
                                vit_all[:, b, t, c * 512:(c + 1) * 512])
                        nc.vector.bn_aggr(st2a[:, t, :], st6[:, :, :])
                    sda = small.tile([128, 4], F32, tag="sda")
                    nc.scalar.activation(sda[:, :], st2a[:, :, 1], AF.Sqrt,
                                         bias=eps_t[:, :], scale=1.0)
                    ra = small.tile([128, 4], F32, tag="ra")
                    nc.vector.reciprocal(ra[:, :], sda[:, :])
                    nmra = small.tile([128, 4], F32, tag="nmra")
                    nc.vector.scalar_tensor_tensor(
                        nmra[:, :], st2a[:, :, 0], -1.0, ra[:, :],
                        op0=ALU.mult, op1=ALU.mult,
                    )
                for t, (st, w) in enumerate(NT):
                    if ln_batched and t < 4:
                        r, nmr = ra[:, t:t + 1], nmra[:, t:t + 1]
                    else:
                        r, nmr = layernorm_stats(vit_all[:, b, t, :], w, LOW)
                    xh = stageB.tile([128, LOW], F16, tag="xhv", name="xh")
                    nc.gpsimd.tensor_scalar(xh[:w, :], vit_all[:w, b, t, :],
                                            r[:w, :], nmr[:w, :],
                                            op0=ALU.mult, op1=ALU.add)
                    xpose_eng = (nc.scalar
                                 if os.environ.get("BF_XTR", "0") == "1"
                                 else nc.sync)
                    xpose_eng.dma_start_transpose(xT[:, :, st:st + w],
                                                  xh[:w, :])

            # cT[f, l-pad] = Wqk^T @ boxT, batch-concat (feeds phase-B att)
            for ft in range(HT_LOW):
                ps = pp_mm.tile([128, 512], F32, tag="mm", name="ps")
                if c8:
                    for hp in range(HT_HIGH // 2):
                        nc.tensor.matmul(
                            ps[:, :LBP],
                            wqk[:, 2 * hp:2 * hp + 2, ft * 128:(ft + 1) * 128],
                            boxT8[:, 2 * hp:2 * hp + 2, :],
                            start=(hp == 0), stop=(hp == HT_HIGH // 2 - 1),
                            perf_mode=DR,
                        )
                else:
                    for h in range(HT_HIGH):
                        nc.tensor.matmul(
                            ps[:, :LBP], wqk[:, h, ft * 128:(ft + 1) * 128],
                            boxT16[:, h, :],
                            start=(h == 0), stop=(h == HT_HIGH - 1),
                        )
                nc.scalar.activation(cT[:, ft, :], ps[:, :LBP],
                                     AF.Identity, bias=cqb[:, ft:ft + 1],
                                     scale=(1.0 / S_WQK) if c8 else 1.0)

            if has_qb:
                # kqb[l] = x_hat_box . (Wk@qb) -> exp bias per batch
                for b in range(BPC):
                    psb = pp_att.tile([128, 512], F32, tag="att", name="psb")
                    for h in range(HT_HIGH):
                        nc.tensor.matmul(
                            psb[:L, :1], boxT16[:, h, b * LP:b * LP + L],
                            wkqb[:, h:h + 1],
                            start=(h == 0), stop=(h == HT_HIGH - 1))
                    nc.vector.scalar_tensor_tensor(
                        bias_all[:L, b:b + 1], psb[:L, :1], msc[:L, b:b + 1],
                        mbs[:L, b:b + 1], op0=ALU.mult, op1=ALU.add,
                    )

            # ---------------- Phase B: per batch ----------------
            for b in range(BPC):
                xT = xTs[b]
                ebias = bias_all if has_qb else mbs
                # attT[l, n] = cT . xT over f; exp with mask+bias fused
                attT = attp.tile([128, NTOK], F16, tag="attT")
                for cs, cw in CNT:
                    ps = pp_att.tile([128, 512], F32, tag="att", name="ps")
                    for ft in range(HT_LOW):
                        nc.tensor.matmul(
                            ps[:L, :cw], cT[:, ft, b * LP:b * LP + L],
                            xT[:, ft, cs:cs + cw],
                            start=(ft == 0), stop=(ft == HT_LOW - 1),
                        )
                    nc.scalar.activation(attT[:L, cs:cs + cw], ps[:L, :cw],
                                         AF.Exp, bias=ebias[:L, b:b + 1],
                                         scale=msc[:L, b:b + 1])

                # rowsum, reciprocal, att@v, epilogue
                inv = small.tile([128, 5], F32, tag="inv")
                for s, (st, w) in enumerate(NT):
                    pss = pp_att.tile([128, 512], F32, tag="att", name="pss")
                    nc.tensor.matmul(pss[:w, :1], attT[:L, st:st + w],
                                     ones[:L, :], start=True, stop=True)
                    nc.vector.reciprocal(inv[:w, s:s + 1], pss[:w, :1])
                    outst = outp.tile([128, LOW], F16, tag="outst")
                    # BF_PROBE=1: timing-only probe that halves the att@v
                    # matmul stream (WRONG numerics; never ship enabled)
                    probe_skip = os.environ.get("BF_PROBE", "0") == "1"
                    for c in range(2):
                        cs = c * 512
                        psv_pool = (pp_mm
                                    if os.environ.get("BF_PSB", "0") == "1"
                                    else pp_att)
                        psv = psv_pool.tile([128, 512], F32,
                                            tag="mm" if psv_pool is pp_mm
                                            else "att", name="psv")
                        nc.tensor.matmul(
                            psv[:w, :1] if (probe_skip and c == 1) else psv[:w, :],
                            attT[:L, st:st + w],
                            vnat[:L, b, cs:cs + 1] if (probe_skip and c == 1)
                            else vnat[:L, b, cs:cs + 512],
                            start=True, stop=True,
                        )
                        if c == 0 or b == BPC - 1:
                            nc.vector.scalar_tensor_tensor(
                                outst[:w, cs:cs + 512], psv[:w, :],
                                inv[:w, s:s + 1], vit_all[:w, b, s, cs:cs + 512],
                                op0=ALU.mult, op1=ALU.add,
                            )
                        else:
                            # scalar scales+casts, gpsimd adds the residual
                            nc.scalar.activation(
                                outst[:w, cs:cs + 512], psv[:w, :],
                                AF.Identity, scale=inv[:w, s:s + 1],
                            )
                            nc.gpsimd.tensor_tensor(
                                outst[:w, cs:cs + 512], outst[:w, cs:cs + 512],
                                vit_all[:w, b, s, cs:cs + 512], op=ALU.add,
                            )
                    nc.scalar.dma_start(out_d[b, st:st + w, :], outst[:w, :])

    nc.compile()
    return nc


def kernel(**inputs):
    import ml_dtypes
    from concourse.bass_utils import run_bass_kernel_spmd

    F16NP = np.float16
    f32 = np.float32

    vit = np.asarray(inputs["vit_feat"], dtype=f32)
    box = np.asarray(inputs["box_feat"], dtype=f32)
    lengths = np.asarray(inputs["lengths"])

    def eff(ln_w, ln_b, w, bias, scale=1.0):
        w = np.asarray(w, f32)
        weff = (np.asarray(ln_w, f32)[:, None] * w) * f32(scale)
        beff = (np.asarray(ln_b, f32) @ w + np.asarray(bias, f32)) * f32(scale)
        return weff, beff

    att_scale = 1.0 / np.sqrt(np.float32(LOW))
    qw, qbv = eff(inputs["q_ln_w"], inputs["q_ln_b"], inputs["q_w"], inputs["q_b"],
                  att_scale)
    kw, kbv = eff(inputs["k_ln_w"], inputs["k_ln_b"], inputs["k_w"], inputs["k_b"])
    vw, vbv = eff(inputs["v_ln_w"], inputs["v_ln_b"], inputs["v_w"], inputs["v_b"])

    # host folds: c = x_hat_box @ Wqk + cqb;  kqb = x_hat_box @ wkqb + kb.qb
    wqk = kw @ qw.T                      # [HIGH, LOW] f32
    cqbv = qw @ kbv                      # [LOW]
    wkqbv = kw @ qbv                     # [HIGH]
    kqb_const = float(kbv @ qbv)
    has_qb = bool(np.any(qbv != 0.0))

    c8, v8 = _flags()
    FP8NP = ml_dtypes.float8_e4m3  # TRN FP8_EXP4-compatible (max +-240)

    # layouts: [p, h, d] = W[h*128+p, d]
    def pack_w(w, use8, scale):
        wl = np.ascontiguousarray(w.reshape(HT_HIGH, 128, LOW).transpose(1, 0, 2))
        if use8:
            return np.clip(wl * f32(scale), -240.0, 240.0).astype(FP8NP)
        return wl.astype(F16NP)

    wqk16 = pack_w(wqk, c8, S_WQK)
    vw16 = pack_w(vw, v8, S_VW)
    cqbL = np.ascontiguousarray(cqbv.reshape(DT, 128).T)

    vit16 = (vit + vbv[None, None, :]).astype(F16NP)   # v bias folded here
    box16 = box.astype(F16NP)

    valid = (np.arange(L)[None, :] < lengths[:, None].astype(np.int64))  # [B, L]
    msc_all = valid.astype(f32)
    mbs_all = np.where(valid, f32(0.0), f32(MASK_NEG))
    if has_qb:
        mbs_all = mbs_all + msc_all * f32(kqb_const)
    ones = np.ones((128, 1), dtype=F16NP)

    key = ("nc", has_qb, c8, v8)
    if key not in _CACHE:
        _CACHE[key] = _build(has_qb=has_qb, c8=c8, v8=v8)
        _CACHE["nc"] = _CACHE[key]
    nc = _CACHE[key]

    in_maps = []
    for c in range(NCORES):
        sl = slice(c * BPC, (c + 1) * BPC)
        m = {
            "vit": np.ascontiguousarray(vit16[sl]),
            "box": np.ascontiguousarray(box16[sl]),
            "wqk": wqk16, "vw": vw16, "cqb": cqbL,
            "msc": np.ascontiguousarray(msc_all[sl].T),
            "mbs": np.ascontiguousarray(mbs_all[sl].T),
            "ones": ones,
        }
        if has_qb:
            m["wkqb"] = np.ascontiguousarray(
                wkqbv.reshape(HT_HIGH, 128).T).astype(F16NP)
        in_maps.append(m)

    _CACHE["in_maps"] = in_maps
    _CACHE["has_qb"] = has_qb
    res = run_bass_kernel_spmd(nc, in_maps, core_ids=list(range(NCORES)))
    out = np.concatenate([np.asarray(res.results[c]["out"]) for c in range(NCORES)],
                         axis=0)
    return np.ascontiguousarray(out.astype(np.float32))


if __name__ == "__main__":
    rng = np.random.default_rng(0)
    ins = {
        "vit_feat": rng.standard_normal((B, NTOK, LOW)).astype(np.float32),
        "box_feat": rng.standard_normal((B, L, HIGH)).astype(np.float32),
        "lengths": rng.integers(0, L, (B,)).astype(np.int64),
        "q_ln_w": np.ones(LOW, np.float32), "q_ln_b": np.zeros(LOW, np.float32),
        "q_w": (rng.standard_normal((LOW, LOW)) * 0.02).astype(np.float32),
        "q_b": np.zeros(LOW, np.float32),
        "k_ln_w": np.ones(HIGH, np.float32), "k_ln_b": np.zeros(HIGH, np.float32),
        "k_w": (rng.standard_normal((HIGH, LOW)) * 0.02).astype(np.float32),
        "k_b": np.zeros(LOW, np.float32),
        "v_ln_w": np.ones(HIGH, np.float32), "v_ln_b": np.zeros(HIGH, np.float32),
        "v_w": (rng.standard_normal((HIGH, LOW)) * 0.02).astype(np.float32),
        "v_b": np.zeros(LOW, np.float32),
    }
    out = kernel(**ins)
    print("out", out.shape, out.dtype, np.abs(out).mean())
